# revision 1
# baseline (speedup 1.0000x reference)
"""Trainium2 Bass kernel for nn_CustomLoss_17875653886357.

Contrastive-style loss vs. the last row (anchor) of the batch:
    lab  = (labels != labels[-1])                        [N]
    dist = ||coords - coords[-1]||^2                     [N]
    loss = sum((1-lab)*dist + lab*max(0, MARGIN-dist))   scalar

Sharding: data-parallel over N across 8 NeuronCores (4096 rows each).
The anchor row (3 floats + 1 int) is baked into the compiled kernel as
immediates; each core produces a scalar partial sum; host adds the 8
partials (the gather/unshard step).

Raw Bacc (no Tile framework): the kernel is a straight pipeline, so
hand-placed semaphores avoid Tile's entry branches and tail
drain+barrier+sem-clear sequence (~1.5us of a ~15us kernel).
"""

from contextlib import ExitStack

import numpy as np

import concourse.mybir as mybir
from concourse import bacc
from concourse.bass_utils import run_bass_kernel_spmd

N, D = 32768, 3
NCORES = 8
NS = N // NCORES  # rows per core = 4096
P = 128  # SBUF partitions
M = NS // P  # rows per partition = 32
MARGIN = 500.0

F32 = mybir.dt.float32
I32 = mybir.dt.int32
Alu = mybir.AluOpType


def _build(anchor_pt, anchor_lab):
    """Build the per-core Bacc program. Anchor values are compile-time
    immediates (the kernel is compiled per call, after inputs are known)."""
    ax, ay, az = (float(v) for v in anchor_pt)
    al = float(int(anchor_lab))  # labels are small ints; exact in f32

    nc = bacc.Bacc(
        "TRN2", target_bir_lowering=False, debug=False, enable_partition_id=False
    )
    bb = nc.cur_bb.bb
    init_names = {i.name for i in bb.instructions}
    coords_d = nc.declare_dram_parameter("coords", [P, M * D], F32, isOutput=False)
    labels_d = nc.declare_dram_parameter("labels", [P, M], I32, isOutput=False)
    out_d = nc.declare_dram_parameter("out", [1, 1], F32, isOutput=True)

    with ExitStack() as ctx:
        C = ctx.enter_context(nc.sbuf_tensor("C", [P, M * D], F32))
        L = ctx.enter_context(nc.sbuf_tensor("L", [P, M], I32))
        AB = ctx.enter_context(nc.sbuf_tensor("AB", [P, M * D], F32))
        E = ctx.enter_context(nc.sbuf_tensor("E", [P, M], F32))
        DIFF = ctx.enter_context(nc.sbuf_tensor("DIFF", [P, M * D], F32))
        SQ = ctx.enter_context(nc.sbuf_tensor("SQ", [P, M * D], F32))
        DN = ctx.enter_context(nc.sbuf_tensor("DN", [P, M], F32))
        H = ctx.enter_context(nc.sbuf_tensor("H", [P, M], F32))
        B = ctx.enter_context(nc.sbuf_tensor("B", [P, M], F32))
        EM = ctx.enter_context(nc.sbuf_tensor("EM", [P, M], F32))
        LOSS = ctx.enter_context(nc.sbuf_tensor("LOSS", [P, M], F32))
        RS = ctx.enter_context(nc.sbuf_tensor("RS", [P, 1], F32))
        ONES = ctx.enter_context(nc.sbuf_tensor("ONES", [P, 1], F32))
        ACC = ctx.enter_context(nc.psum_tensor("ACC", [1, 1], F32))
        cin_sem = ctx.enter_context(nc.semaphore("cin_sem"))
        lin_sem = ctx.enter_context(nc.semaphore("lin_sem"))
        v_sem = ctx.enter_context(nc.semaphore("v_sem"))
        pe_sem = ctx.enter_context(nc.semaphore("pe_sem"))
        out_sem = ctx.enter_context(nc.semaphore("out_sem"))

        # Both input DMAs on sync's HW queue, coords first: all 16 phys
        # DMA engines chew the 128 coords descriptors in ~0.5us, and a
        # single queue guarantees coords always beat labels to SBUF (a
        # second engine's queue can win the DMA-pool race and stall the
        # coords transfer behind the labels one).
        dma_a = nc.sync.dma_start(C[:], coords_d[:])
        dma_a.then_inc(cin_sem, 16)
        dma_b = nc.sync.dma_start(L[:], labels_d[:])
        dma_b.then_inc(lin_sem, 16)

        # DVE instructions don't interlock with their predecessors' writes
        # (deep pipeline), so every same-engine RAW needs a semaphore hop:
        # each op bumps v_sem, dependent ops wait for the producer's count.
        vs = [0]

        def vop(inst):
            inst.then_inc(v_sem, 1)
            vs[0] += 1
            return vs[0]

        # --- constants on DVE while the DMAs fly
        AB3 = AB[:].rearrange("p (m d) -> p m d", d=D)
        vop(nc.vector.memset(AB3[:, :, 0], ax))
        vop(nc.vector.memset(AB3[:, :, 1], ay))
        vop(nc.vector.memset(AB3[:, :, 2], az))
        ones_t = vop(nc.vector.memset(ONES[:], 1.0))

        # --- dist path (needs both coords halves)
        nc.vector.wait_ge(cin_sem, 16)
        nc.vector.wait_ge(v_sem, 3)  # AB memsets retired
        vop(nc.vector.tensor_sub(DIFF[:], C[:], AB[:]))
        nc.vector.wait_ge(v_sem, vs[0])
        vop(nc.vector.tensor_tensor(SQ[:], DIFF[:], DIFF[:], Alu.mult))
        SQ3 = SQ[:].rearrange("p (m d) -> p m d", d=D)
        nc.vector.wait_ge(v_sem, vs[0])
        vop(
            nc.vector.tensor_reduce(  # DN = -dist
                DN[:], SQ3, axis=mybir.AxisListType.X, op=Alu.add, negate=True
            )
        )
        # H = max(MARGIN - dist, 0) = max(DN + MARGIN, 0)
        nc.vector.wait_ge(v_sem, vs[0])
        h_t = vop(nc.vector.tensor_scalar(H[:], DN[:], MARGIN, 0.0, Alu.add, Alu.max))

        # label path, slotted here: no same-engine dependency, so it
        # overlaps the H->B semaphore hop (labels landed long ago)
        nc.vector.wait_ge(lin_sem, 16)
        vop(nc.vector.tensor_scalar(E[:], L[:], int(al), None, Alu.is_equal))

        nc.vector.wait_ge(v_sem, h_t)
        vop(nc.vector.tensor_add(B[:], DN[:], H[:]))  # B = H - dist

        # loss = H - E*B;  RS = per-partition sum(loss)
        nc.vector.wait_ge(v_sem, vs[0])
        vop(nc.vector.tensor_tensor(EM[:], E[:], B[:], Alu.mult))
        nc.vector.wait_ge(v_sem, vs[0])
        rs_t = vop(
            nc.vector.scalar_tensor_tensor(
                LOSS[:], EM[:], -1.0, H[:], Alu.mult, Alu.add, accum_out=RS[:]
            )
        )

        # --- cross-partition reduction on PE: [1,1] = RS.T @ ones
        nc.tensor.wait_ge(v_sem, rs_t)
        nc.tensor.matmul(ACC[:], RS[:], ONES[:], start=True, stop=True).then_inc(
            pe_sem, 1
        )

        # --- result: PSUM -> SBUF (DMA cannot read PSUM), then DMA out
        OUT = ctx.enter_context(nc.sbuf_tensor("OUT", [1, 1], F32))
        nc.vector.wait_ge(pe_sem, 1)
        out_t = vop(nc.vector.tensor_copy(OUT[:], ACC[:]))
        nc.sync.wait_ge(v_sem, out_t)
        # No completion wait here: the NEFF runtime epilogue drains every
        # engine's DMA queues before signalling completion, which covers
        # this last transfer.
        nc.sync.dma_start(out_d[:], OUT[:], single_packet=True).then_inc(out_sem, 16)

    # Bass.__init__ emits per-engine const-tile memsets plus a full
    # drain + all-engine barrier. This kernel uses none of the const
    # tiles, and the NEFF runtime prologue already syncs all engines, so
    # drop them. Also hoist the two HWDGE coords DMAs to the very front:
    # they carry pure access patterns (no registers), so they need not
    # sit behind the ~1.3us per-engine TPB-base loads — issuing first
    # hides that latency behind the DMA flight time.
    strip = {
        i.name
        for i in bb.instructions
        if i.name in init_names
        and type(i).__name__ in ("InstMemset", "InstDrain", "InstEventSemaphore")
    }
    front_names = {dma_a.ins.name, dma_b.ins.name}
    kept = [i for i in bb.instructions if i.name not in strip]
    front = [i for i in kept if i.name in front_names]
    rest = [i for i in kept if i.name not in front_names]
    idx = next(k for k, i in enumerate(rest) if i.name.endswith("dummycall")) + 1
    bb.instructions[:] = rest[:idx] + front + rest[idx:]

    nc.compile()
    return nc


_nc_cache = {}


def build_nc_and_inmaps(batched_labels, batched_predicted_coords):
    labels = np.ascontiguousarray(batched_labels)
    coords = np.ascontiguousarray(batched_predicted_coords, dtype=np.float32)
    assert labels.shape == (N,) and coords.shape == (N, D)
    if labels.dtype != np.int32:
        labels = labels.astype(np.int32)

    key = (coords[-1].tobytes(), int(labels[-1]))
    nc = _nc_cache.get(key)
    if nc is None:
        nc = _nc_cache[key] = _build(coords[-1], labels[-1])

    in_maps = []
    for i in range(NCORES):
        sl = slice(i * NS, (i + 1) * NS)
        in_maps.append(
            {
                "coords": np.ascontiguousarray(coords[sl]).reshape(P, M * D),
                "labels": np.ascontiguousarray(labels[sl]).reshape(P, M),
            }
        )
    return nc, in_maps


def kernel(batched_labels, batched_predicted_coords, _trace=False, _results=[None]):
    nc, in_maps = build_nc_and_inmaps(batched_labels, batched_predicted_coords)
    res = run_bass_kernel_spmd(nc, in_maps, core_ids=list(range(NCORES)), trace=_trace)
    _results[0] = res
    total = np.float64(0.0)
    for r in res.results:
        total += np.float64(r["out"][0, 0])
    return np.array(np.float32(total))

